# revision 42
# baseline (speedup 1.0000x reference)
"""Trainium2 Bass kernel for nn_ExpertClassifierBank.

Computes, for pooled [B,K,D], expert weights [E,C,D], indices [K], log_scales [E]:
    x = l2norm(pooled, axis=-1)
    w = l2norm(weights[idx], axis=-1)
    out[b,k,c] = min(exp(log_scales[idx[k]]), 100) * dot(x[b,k], w[k,c])

Sharding: data-parallel over batch B across 8 NeuronCores (512 rows each);
the gathered expert weight bank is replicated.

Device algorithm per core (B_loc=512, K=8, D=1024=8x128, C=100):
  - host pre-transposes x and w to [k, d-part, j, *] bf16 tiles so the
    contraction dim d sits on SBUF partitions; host also folds the tiny
    per-expert constants into rwt[c,k] = min(exp(ls_k),100)/||w_k,c||.
  - main matmuls produce lg[k] = w_k^T x in PSUM; the drain to SBUF applies
    rwt as a per-partition scale (ACT Copy); the x-normalizer
    f = rsqrt(||x||^2) is broadcast over C partitions by a tiny selector
    matmul (selc4, bf16) and multiplied in on the DVE with bf16 output.
  - row sums-of-squares: bf16 squares (split DVE/ACT) stream through the
    PE via 8-WAY TILED packs: contraction split into 2 row-groups (64
    rows each) x 4 col-groups, tile_position=(64r, 32g) with g=j%4,
    r=j//4, so all 8 ss matmuls of one k issue back-to-back and execute
    concurrently. Group (r,g) partials land in PSUM rows 32g+4r+i of one
    [128,BLOC] bank per half; a bf16 copy + tiny combine matmul (selcmb
    [128,4]) reduces the 8 partials -> [4,BLOC]; recip+sqrt give f.
    Half 1 combines TWICE: after k6 (slots 0-2 complete -> outputs 4-6
    drain mid-stream) and after k7 (slot 3 -> only out7 in the tail).
  - DMA: sync HWDGE queue carries x0 pieces then [w_k, x_k halves] per k
    (w just ahead of its mains) and out7 at the tail; gpsimd SWDGE
    carries the tiny consts and outputs 0-6.
"""

import time

import numpy as np
import ml_dtypes

import concourse.bass as bass
import concourse.mybir as mybir
import concourse.tile as tile
from concourse import bacc
from concourse.bass_utils import run_bass_kernel_spmd

N_CORES = 8
B, K, D, C, E = 4096, 8, 1024, 100, 16
BLOC = B // N_CORES  # 512
P = 128
DC = D // P  # 8 d-chunks
HALF = 4  # k-batch size for the f pipeline

F32 = mybir.dt.float32
BF16 = mybir.dt.bfloat16
AF = mybir.ActivationFunctionType
NPBF16 = ml_dtypes.bfloat16

_CACHE = {}

LAST_RESULT = None
LAST_WALL_NS = None


def _build():
    nc = bacc.Bacc(
        "TRN2", target_bir_lowering=False, debug=False, num_devices=N_CORES
    )

    xt = nc.dram_tensor("xt", [K, P, DC, BLOC], BF16, kind="ExternalInput").ap()
    wt = nc.dram_tensor("wt", [K, P, DC, C], BF16, kind="ExternalInput").ap()
    selk8 = nc.dram_tensor("selk8", [P, HALF, HALF], BF16, kind="ExternalInput").ap()
    selc4 = nc.dram_tensor("selc4", [HALF, HALF, C], BF16, kind="ExternalInput").ap()
    selcmb = nc.dram_tensor("selcmb", [P, HALF], BF16, kind="ExternalInput").ap()
    out = nc.dram_tensor("out", [K, C, BLOC], BF16, kind="ExternalOutput").ap()
    # DRAM bounce for the f values: a 0-stride partition broadcast is only
    # legal on a DRAM source, so fx is stored once per phase and each k's
    # broadcast-load replicates row i over C partitions
    fxd = nc.dram_tensor("fxd", [2, HALF, BLOC], BF16, kind="Internal").ap()

    with tile.TileContext(nc) as tc:
        with (
            tc.tile_pool(name="const", bufs=1) as cpool,
            tc.tile_pool(name="xres", bufs=K) as xpool,
            tc.tile_pool(name="x2", bufs=3) as x2pool,
            tc.tile_pool(name="wres", bufs=K) as wpool,
            tc.tile_pool(name="small", bufs=1) as spool,
            tc.tile_pool(name="lgs", bufs=5) as lgspool,
            tc.tile_pool(name="osb", bufs=8) as opool,
            tc.tile_pool(name="fx", bufs=2) as fxpool,
            tc.tile_pool(name="sscp", bufs=2) as sscppool,
            tc.tile_pool(name="fbx", bufs=3) as fbxpool,
        ):
            # dummy first ACT op: pulls the (single) table set load to t~0
            dum = spool.tile([1, 1], F32, name="dum")
            nc.vector.memset(dum[:], 1.0)
            dum2 = spool.tile([1, 1], F32, name="dum2")
            nc.scalar.activation(dum2[:], dum[:], AF.Square)

            # ---- sync HWDGE queue: x0 pieces first, then per k the w
            # tile just ahead of the x halves it gates ----
            x_sbs = [None] * K
            w_sbs = [None] * K

            x0 = xpool.tile([P, DC, BLOC], BF16, tag="x", name="x0")
            x_sbs[0] = x0
            w0 = wpool.tile([P, DC, C], BF16, tag="w", name="w0")
            w_sbs[0] = w0
            nc.sync.dma_start(w0[:, :2], wt[0][:, :2])
            nc.sync.dma_start(x0[:, :2], xt[0][:, :2])
            nc.sync.dma_start(w0[:, 2:], wt[0][:, 2:])
            nc.sync.dma_start(x0[:, 2:], xt[0][:, 2:])
            for k in range(1, K):
                w_sb = wpool.tile([P, DC, C], BF16, tag="w", name=f"w{k}")
                w_sbs[k] = w_sb
                nc.sync.dma_start(w_sb[:], wt[k])
                x_sb = xpool.tile([P, DC, BLOC], BF16, tag="x", name=f"x{k}")
                x_sbs[k] = x_sb
                nc.sync.dma_start(x_sb[:, : DC // 2], xt[k][:, : DC // 2])
                nc.sync.dma_start(x_sb[:, DC // 2 :], xt[k][:, DC // 2 :])

            # ---- tiny consts on the gpsimd SWDGE queue ----
            selk8_sb = cpool.tile([P, HALF, HALF], BF16)
            nc.gpsimd.dma_start(selk8_sb[:], selk8[:])
            selcmb_sb = cpool.tile([P, HALF], BF16)
            nc.gpsimd.dma_start(selcmb_sb[:], selcmb[:])
            selc4_sb = cpool.tile([HALF, HALF, C], BF16)
            nc.gpsimd.dma_start(selc4_sb[:], selc4[:])

            with (
                tc.tile_pool(name="pss", bufs=2, space="PSUM") as pss,
                tc.tile_pool(name="plog", bufs=3, space="PSUM") as plog,
                tc.tile_pool(name="pfb", bufs=1, space="PSUM") as pfb,
                tc.tile_pool(name="pfx", bufs=1, space="PSUM") as pfx,
            ):
                sss = []        # per-half [P, BLOC] group-partial PSUM
                fx_sbs = {}     # combine-phase -> [HALF, BLOC] bf16 f
                lg_tiles = {}
                x2s = {}
                ss_started = set()

                for half in range(2):
                    ss_t = pss.tile([P, BLOC], F32, tag="ss",
                                    name=f"ss{half}")
                    # zero the bank: rows outside 32g+4r+i are never
                    # written by the ss matmuls but ARE read (x0 weight)
                    # by the combine matmul via the bf16 copy -- garbage
                    # there can be NaN and 0*NaN = NaN. has_written stays
                    # unset, so the first matmul write still overwrites.
                    nc.vector.memset(ss_t[:], 0.0)
                    sss.append(ss_t)

                def emit_drain(k):
                    # plain PSUM->SBUF copy (rwt folded into w host-side);
                    # bf16 so the output mul runs in DVE 2x mode
                    lgs = lgspool.tile([C, BLOC], BF16, tag="lgs",
                                       name=f"lgs{k}")
                    nc.scalar.activation(lgs[:], lg_tiles[k][:], AF.Copy)
                    lg_tiles[k] = lgs

                def emit_output_stage(k, phase):
                    """gpsimd f-broadcast + bf16 mul + out DMA (fx ready).

                    The broadcast replaces a PE matmul: fx row i is
                    replicated over C partitions by GpSimd, freeing the
                    tensor engine (the steady-state bottleneck), and the
                    mul becomes bf16xbf16 SBUF (DVE 2x mode).
                    """
                    i = k % HALF
                    pidx = 0 if phase == "a" else 1
                    fbx = fbxpool.tile([C, BLOC], BF16, tag="fbx",
                                       name=f"fbx{k}")
                    nc.gpsimd.dma_start(
                        fbx[:],
                        fxd[pidx][i : i + 1, :].to_broadcast((C, BLOC)),
                    )
                    o_sb = opool.tile([C, BLOC], BF16, tag="osb", name=f"o{k}")
                    nc.vector.tensor_mul(o_sb[:], lg_tiles[k][:], fbx[:])
                    nc.gpsimd.dma_start(out[k], o_sb[:])

                def emit_squares(k, p0=0, p1=DC // 2):
                    """Square chunk-pairs [p0,p1) of x_k into x2 (bf16)."""
                    if p0 == 0:
                        x2s[k] = x2pool.tile([P, DC, BLOC], BF16, tag="x2",
                                             name=f"x2_{k}")
                    x2 = x2s[k]
                    for p in range(p0, p1):
                        src = x_sbs[k][:, 2 * p : 2 * p + 2]
                        dst = x2[:, 2 * p : 2 * p + 2]
                        # measured: DVE ~594ns/op, ACT ~1040-1360ns/op,
                        # gpsimd ~2120ns/op (too slow, and it stalled the
                        # ss packs) -- so 1 op to ACT, 3 to DVE
                        if p == 1:
                            nc.scalar.activation(dst, src, AF.Square)
                        else:
                            nc.vector.tensor_mul(dst, src, src)

                def mains(k, j0, j1):
                    for j in range(j0, j1):
                        nc.tensor.matmul(
                            lg_tiles[k][:], lhsT=w_sbs[k][:, j, :],
                            rhs=x_sbs[k][:, j],
                            start=(j == 0), stop=(j == DC - 1),
                            skip_group_check=True,
                        )

                def ssmm(k):
                    """Column-tiled ss pack: group g=j%4 in PE col-group g
                    (tile_position=(0,32g)); waves of 4 run concurrently.
                    (Row-splitting the contraction for 8-way was tried and
                    crashes the device: concurrent row-tiles must drain to
                    DIFFERENT PSUM banks, and all groups share one here.)

                    All chains share one PSUM bank, so only the first
                    matmul issued into the bank carries start=True (the
                    bank-level has_written clear); later first-writes per
                    element overwrite via the unset bit, then accumulate.
                    """
                    half, i = divmod(k, HALF)
                    for j in range(DC):
                        g = j % HALF
                        first = half not in ss_started
                        ss_started.add(half)
                        nc.tensor.matmul(
                            sss[half][32 * g : 32 * g + HALF, :],
                            lhsT=selk8_sb[:, i, :],
                            rhs=x2s[k][:, j],
                            start=first,
                            stop=(i == HALF - 1 and j == DC - 1),
                            skip_group_check=True,
                            tile_position=(0, 32 * g),
                        )

                def emit_combine(half, phase):
                    """bf16 copy + selcmb matmul + recip/sqrt -> fx.

                    Phase c's copy rides the DVE: at the tail the ACT
                    queue sits behind k7's square, and a 2.2us PE stall
                    waiting on the ACT copy was measured.
                    """
                    cp = sscppool.tile([P, BLOC], BF16, tag="sscp",
                                       name=f"sscp{phase}")
                    if phase == "c":
                        nc.vector.tensor_copy(cp[:], sss[half][:])
                    else:
                        nc.scalar.activation(cp[:], sss[half][:], AF.Copy)
                    fxp = pfx.tile([HALF, BLOC], F32, tag="fxp",
                                   name=f"fxp{phase}")
                    nc.tensor.matmul(
                        fxp[:],
                        lhsT=selcmb_sb[:],
                        rhs=cp[:],
                        start=True, stop=True,
                        skip_group_check=True,
                    )
                    recx = fxpool.tile([HALF, BLOC], F32, tag="recx",
                                       name=f"recx{phase}")
                    nc.vector.reciprocal_approx_fast(recx[:], fxp[:])
                    fx = fxpool.tile([HALF, BLOC], BF16, tag="fx",
                                     name=f"fx{phase}")
                    nc.scalar.activation(fx[:], recx[:], AF.Sqrt)
                    fx_sbs[phase] = fx
                    if phase != "c":
                        nc.gpsimd.dma_start(fxd[0 if phase == "a" else 1],
                                            fx[:])

                def new_lg(k):
                    lg_tiles[k] = plog.tile([C, BLOC], F32, tag="lg",
                                            name=f"lg{k}")

                # HAM warmup: ~3.2us of tiny junk matmuls on the const
                # tiles (land ~8.5us) so the real stream runs at 2.4GHz
                junk = pfb.tile([HALF, HALF], F32, tag="junk", name="junk")
                for _ in range(36):
                    nc.tensor.matmul(
                        junk[:], lhsT=selk8_sb[:, 0, :], rhs=selcmb_sb[:],
                        start=True, stop=True, skip_group_check=True,
                    )

                # k=0: mains in DMA-piece order (earliest PE start), then ss
                emit_squares(0, 0, 1)
                new_lg(0)
                mains(0, 0, 2)
                emit_squares(0, 1, 4)
                mains(0, 2, DC)
                ssmm(0)
                for k in range(1, K - 1):
                    emit_squares(k)
                    new_lg(k)
                    mains(k, 0, DC)
                    ssmm(k)
                    emit_drain(k - 1)
                    if k == HALF - 1:
                        emit_combine(0, "a")
                    if k == K - 2:
                        emit_combine(1, "b")
                    if k >= HALF:
                        emit_output_stage(k - HALF, "a")
                    if k == HALF + 1:
                        emit_output_stage(HALF - 1, "a")

                # k=7: ss pack first; the tiny slot-3 combine slots between
                # the mains so only out7's chain trails the last matmul
                k = K - 1
                emit_drain(k - 1)
                emit_squares(k)
                new_lg(k)
                emit_output_stage(HALF, "b")
                emit_output_stage(HALF + 1, "b")
                emit_output_stage(HALF + 2, "b")
                ssmm(k)
                mains(k, 0, 1)
                emit_combine(1, "c")
                mains(k, 1, 6)
                # k7 output off the critical chain: fb7 early (fx_c lands
                # mid-mains), drain the SMALL fb on ACT during the last
                # mains, then mul7 reads lg7 straight from PSUM -- no lg
                # drain in the tail
                fb7 = pfb.tile([C, BLOC], F32, tag="fb", name="fb7")
                nc.tensor.matmul(
                    fb7[:], lhsT=selc4_sb[:, HALF - 1, :],
                    rhs=fx_sbs["c"][:], start=True, stop=True,
                    skip_group_check=True,
                )
                fbd = opool.tile([C, BLOC], BF16, tag="osb", name="fbd7")
                nc.scalar.activation(fbd[:], fb7[:], AF.Copy)
                mains(k, 6, DC)
                o7 = opool.tile([C, BLOC], BF16, tag="osb", name="o7")
                nc.vector.tensor_mul(o7[:], lg_tiles[k][:], fbd[:])
                nc.sync.dma_start(out[k], o7[:])

    nc.compile()
    return nc


def _host_prep(pooled, active_expert_indices, weights, log_scales):
    idx = np.asarray(active_expert_indices).astype(np.int64)
    pooled = np.asarray(pooled, dtype=np.float32)
    weights = np.asarray(weights, dtype=np.float32)
    log_scales = np.asarray(log_scales, dtype=np.float32)

    # x: [B,K,D] -> bf16 -> per-core [K, P, DC, BLOC]  (k, d, j, b)
    pb = pooled.astype(NPBF16)
    xt_all = np.ascontiguousarray(
        pb.reshape(N_CORES, BLOC, K, DC, P).transpose(0, 2, 4, 3, 1)
    )
    # w: gather -> bf16 -> fold min(exp(ls),100)/||w_bf16|| into the
    # weights themselves (re-rounded to bf16), so the drain needs no scale
    wg = weights[idx].astype(NPBF16)  # [K, C, D]
    s = np.minimum(np.exp(log_scales[idx]), 100.0).astype(np.float32)
    wn = np.sqrt((wg.astype(np.float32) ** 2).sum(-1))  # [K, C]
    rw = s[:, None] / np.maximum(wn, 1e-12)  # [K, C]
    wg = (wg.astype(np.float32) * rw[:, :, None]).astype(NPBF16)
    wt = np.ascontiguousarray(wg.reshape(K, C, DC, P).transpose(0, 3, 2, 1))

    # ss selector: one-hot k-slot column, full 128-row contraction
    selk8 = np.zeros((P, HALF, HALF), NPBF16)
    for i in range(HALF):
        selk8[:, i, i] = 1.0
    # combine: column i sums PSUM rows {32g + i}
    selcmb = np.zeros((P, HALF), NPBF16)
    for g in range(HALF):
        for i in range(HALF):
            selcmb[32 * g + i, i] = 1.0
    selc4 = np.zeros((HALF, HALF, C), NPBF16)
    for i in range(HALF):
        selc4[i, i, :] = 1.0

    shared = {"wt": wt, "selk8": selk8, "selc4": selc4, "selcmb": selcmb}
    return [dict(shared, xt=np.ascontiguousarray(xt_all[co]))
            for co in range(N_CORES)]


def kernel(pooled, active_expert_indices, weights, log_scales):
    global LAST_RESULT, LAST_WALL_NS
    if "nc" not in _CACHE:
        _CACHE["nc"] = _build()
    nc = _CACHE["nc"]

    in_maps = _host_prep(pooled, active_expert_indices, weights, log_scales)

    t0 = time.perf_counter_ns()
    res = run_bass_kernel_spmd(nc, in_maps, core_ids=list(range(N_CORES)))
    LAST_WALL_NS = time.perf_counter_ns() - t0
    LAST_RESULT = res

    full = np.stack([res.results[co]["out"] for co in range(N_CORES)])
    return np.ascontiguousarray(
        full.transpose(0, 3, 1, 2).reshape(B, K, C)
    ).astype(np.float32)


# revision 47
# speedup vs baseline: 1.1196x; 1.1196x over previous
"""Trainium2 Bass kernel for nn_ExpertClassifierBank.

Computes, for pooled [B,K,D], expert weights [E,C,D], indices [K], log_scales [E]:
    x = l2norm(pooled, axis=-1)
    w = l2norm(weights[idx], axis=-1)
    out[b,k,c] = min(exp(log_scales[idx[k]]), 100) * dot(x[b,k], w[k,c])

Sharding: data-parallel over batch B across 8 NeuronCores (512 rows each);
the gathered expert weight bank is replicated.

Device algorithm per core (B_loc=512, K=8, D=1024=8x128, C=100):
  - host pre-transposes x and w to [k, d-part, j, *] bf16 tiles so the
    contraction dim d sits on SBUF partitions; host also folds the tiny
    per-expert constants into rwt[c,k] = min(exp(ls_k),100)/||w_k,c||.
  - main matmuls produce lg[k] = w_k^T x in PSUM; the drain to SBUF applies
    rwt as a per-partition scale (ACT Copy); the x-normalizer
    f = rsqrt(||x||^2) is broadcast over C partitions by a tiny selector
    matmul (selc4, bf16) and multiplied in on the DVE with bf16 output.
  - row sums-of-squares: bf16 squares (split DVE/ACT) stream through the
    PE via 8-WAY TILED packs: contraction split into 2 row-groups (64
    rows each) x 4 col-groups, tile_position=(64r, 32g) with g=j%4,
    r=j//4, so all 8 ss matmuls of one k issue back-to-back and execute
    concurrently. Group (r,g) partials land in PSUM rows 32g+4r+i of one
    [128,BLOC] bank per half; a bf16 copy + tiny combine matmul (selcmb
    [128,4]) reduces the 8 partials -> [4,BLOC]; recip+sqrt give f.
    Half 1 combines TWICE: after k6 (slots 0-2 complete -> outputs 4-6
    drain mid-stream) and after k7 (slot 3 -> only out7 in the tail).
  - DMA: sync HWDGE queue carries x0 pieces then [w_k, x_k halves] per k
    (w just ahead of its mains) and out7 at the tail; gpsimd SWDGE
    carries the tiny consts and outputs 0-6.
"""

import time

import numpy as np
import ml_dtypes

import concourse.bass as bass
import concourse.mybir as mybir
import concourse.tile as tile
from concourse import bacc
from concourse.bass_utils import run_bass_kernel_spmd

N_CORES = 8
B, K, D, C, E = 4096, 8, 1024, 100, 16
BLOC = B // N_CORES  # 512
P = 128
DC = D // P  # 8 d-chunks
HALF = 4  # k-batch size for the f pipeline

F32 = mybir.dt.float32
BF16 = mybir.dt.bfloat16
AF = mybir.ActivationFunctionType
NPBF16 = ml_dtypes.bfloat16

_CACHE = {}

LAST_RESULT = None
LAST_WALL_NS = None


def _build():
    nc = bacc.Bacc(
        "TRN2", target_bir_lowering=False, debug=False, num_devices=N_CORES
    )

    xt = nc.dram_tensor("xt", [K, P, DC, BLOC], BF16, kind="ExternalInput").ap()
    wt = nc.dram_tensor("wt", [K, P, DC, C], BF16, kind="ExternalInput").ap()
    selk8 = nc.dram_tensor("selk8", [P, HALF, HALF], BF16, kind="ExternalInput").ap()
    selc4 = nc.dram_tensor("selc4", [HALF, HALF, C], BF16, kind="ExternalInput").ap()
    selcmb = nc.dram_tensor("selcmb", [P, HALF], BF16, kind="ExternalInput").ap()
    out = nc.dram_tensor("out", [K, C, BLOC], BF16, kind="ExternalOutput").ap()

    with tile.TileContext(nc) as tc:
        with (
            tc.tile_pool(name="const", bufs=1) as cpool,
            tc.tile_pool(name="xres", bufs=K) as xpool,
            tc.tile_pool(name="x2", bufs=3) as x2pool,
            tc.tile_pool(name="wres", bufs=K) as wpool,
            tc.tile_pool(name="small", bufs=1) as spool,
            tc.tile_pool(name="lgs", bufs=5) as lgspool,
            tc.tile_pool(name="osb", bufs=8) as opool,
            tc.tile_pool(name="fx", bufs=2) as fxpool,
            tc.tile_pool(name="sscp", bufs=2) as sscppool,
        ):
            # dummy first ACT op: pulls the (single) table set load to t~0
            dum = spool.tile([1, 1], F32, name="dum")
            nc.vector.memset(dum[:], 1.0)
            dum2 = spool.tile([1, 1], F32, name="dum2")
            nc.scalar.activation(dum2[:], dum[:], AF.Square)

            # ---- sync HWDGE queue: x0 pieces first, then per k the w
            # tile just ahead of the x halves it gates ----
            x_sbs = [None] * K
            w_sbs = [None] * K

            x0 = xpool.tile([P, DC, BLOC], BF16, tag="x", name="x0")
            x_sbs[0] = x0
            w0 = wpool.tile([P, DC, C], BF16, tag="w", name="w0")
            w_sbs[0] = w0
            nc.sync.dma_start(w0[:, :2], wt[0][:, :2])
            nc.sync.dma_start(x0[:, :2], xt[0][:, :2])
            nc.sync.dma_start(w0[:, 2:], wt[0][:, 2:])
            nc.sync.dma_start(x0[:, 2:], xt[0][:, 2:])
            for k in range(1, K):
                w_sb = wpool.tile([P, DC, C], BF16, tag="w", name=f"w{k}")
                w_sbs[k] = w_sb
                nc.sync.dma_start(w_sb[:], wt[k])
                x_sb = xpool.tile([P, DC, BLOC], BF16, tag="x", name=f"x{k}")
                x_sbs[k] = x_sb
                nc.sync.dma_start(x_sb[:, : DC // 2], xt[k][:, : DC // 2])
                nc.sync.dma_start(x_sb[:, DC // 2 :], xt[k][:, DC // 2 :])

            # ---- tiny consts on the gpsimd SWDGE queue ----
            selk8_sb = cpool.tile([P, HALF, HALF], BF16)
            nc.gpsimd.dma_start(selk8_sb[:], selk8[:])
            selcmb_sb = cpool.tile([P, HALF], BF16)
            nc.gpsimd.dma_start(selcmb_sb[:], selcmb[:])
            selc4_sb = cpool.tile([HALF, HALF, C], BF16)
            nc.gpsimd.dma_start(selc4_sb[:], selc4[:])

            with (
                tc.tile_pool(name="pss", bufs=2, space="PSUM") as pss,
                tc.tile_pool(name="plog", bufs=3, space="PSUM") as plog,
                tc.tile_pool(name="pfb", bufs=1, space="PSUM") as pfb,
                tc.tile_pool(name="pfx", bufs=1, space="PSUM") as pfx,
            ):
                sss = []        # per-half [P, BLOC] group-partial PSUM
                fx_sbs = {}     # combine-phase -> [HALF, BLOC] bf16 f
                lg_tiles = {}
                x2s = {}
                ss_started = set()

                for half in range(2):
                    ss_t = pss.tile([P, BLOC], F32, tag="ss",
                                    name=f"ss{half}")
                    # zero the bank: rows outside 32g+4r+i are never
                    # written by the ss matmuls but ARE read (x0 weight)
                    # by the combine matmul via the bf16 copy -- garbage
                    # there can be NaN and 0*NaN = NaN. has_written stays
                    # unset, so the first matmul write still overwrites.
                    nc.vector.memset(ss_t[:], 0.0)
                    sss.append(ss_t)

                def emit_drain(k):
                    # plain PSUM->SBUF copy (rwt folded into w host-side);
                    # bf16 so the output mul runs in DVE 2x mode
                    lgs = lgspool.tile([C, BLOC], BF16, tag="lgs",
                                       name=f"lgs{k}")
                    nc.scalar.activation(lgs[:], lg_tiles[k][:], AF.Copy)
                    lg_tiles[k] = lgs

                def emit_output_stage(k, phase, nrows=HALF):
                    """f-broadcast matmul + mul + out DMA (fx ready).

                    nrows<4 restricts the contraction: phase-b fx row 3
                    is rsqrt(0)=inf (slot 3 incomplete) and 0*inf = NaN
                    if streamed, so those fbs contract rows 0-2 only.
                    (A gpsimd/DMA broadcast instead of the fb matmul was
                    tried: the f-chain latency grows ~5us and the output
                    mul head-of-line-blocks the DVE queue -- net loss.)
                    """
                    i = k % HALF
                    fb = pfb.tile([C, BLOC], F32, tag="fb", name=f"fb{k}")
                    nc.tensor.matmul(
                        fb[:],
                        lhsT=selc4_sb[:nrows, i, :],
                        rhs=fx_sbs[phase][:nrows, :],
                        start=True, stop=True,
                        skip_group_check=True,
                    )
                    o_sb = opool.tile([C, BLOC], BF16, tag="osb", name=f"o{k}")
                    nc.vector.tensor_mul(o_sb[:], lg_tiles[k][:], fb[:])
                    nc.gpsimd.dma_start(out[k], o_sb[:])

                def emit_squares(k, p0=0, p1=DC // 2):
                    """Square chunk-pairs [p0,p1) of x_k into x2 (bf16)."""
                    if p0 == 0:
                        x2s[k] = x2pool.tile([P, DC, BLOC], BF16, tag="x2",
                                             name=f"x2_{k}")
                    x2 = x2s[k]
                    for p in range(p0, p1):
                        src = x_sbs[k][:, 2 * p : 2 * p + 2]
                        dst = x2[:, 2 * p : 2 * p + 2]
                        # measured: DVE ~594ns/op, ACT ~1040-1360ns/op,
                        # gpsimd ~2120ns/op (too slow, and it stalled the
                        # ss packs) -- so 1 op to ACT, 3 to DVE
                        if p == 1:
                            nc.scalar.activation(dst, src, AF.Square)
                        else:
                            nc.vector.tensor_mul(dst, src, src)

                def mains(k, j0, j1):
                    for j in range(j0, j1):
                        nc.tensor.matmul(
                            lg_tiles[k][:], lhsT=w_sbs[k][:, j, :],
                            rhs=x_sbs[k][:, j],
                            start=(j == 0), stop=(j == DC - 1),
                            skip_group_check=True,
                        )

                def ssmm(k):
                    """Column-tiled ss pack: group g=j%4 in PE col-group g
                    (tile_position=(0,32g)); waves of 4 run concurrently.
                    (Row-splitting the contraction for 8-way was tried and
                    crashes the device: concurrent row-tiles must drain to
                    DIFFERENT PSUM banks, and all groups share one here.)

                    All chains share one PSUM bank, so only the first
                    matmul issued into the bank carries start=True (the
                    bank-level has_written clear); later first-writes per
                    element overwrite via the unset bit, then accumulate.
                    """
                    half, i = divmod(k, HALF)
                    for j in range(DC):
                        g = j % HALF
                        first = half not in ss_started
                        ss_started.add(half)
                        nc.tensor.matmul(
                            sss[half][32 * g : 32 * g + HALF, :],
                            lhsT=selk8_sb[:, i, :],
                            rhs=x2s[k][:, j],
                            start=first,
                            stop=(i == HALF - 1 and j == DC - 1),
                            skip_group_check=True,
                            tile_position=(0, 32 * g),
                        )

                def emit_combine(half, phase):
                    """bf16 copy + selcmb matmul + recip/sqrt -> fx.

                    Phase c's copy rides the DVE: at the tail the ACT
                    queue sits behind k7's square, and a 2.2us PE stall
                    waiting on the ACT copy was measured.
                    """
                    cp = sscppool.tile([P, BLOC], BF16, tag="sscp",
                                       name=f"sscp{phase}")
                    if phase == "c":
                        nc.vector.tensor_copy(cp[:], sss[half][:])
                    else:
                        nc.scalar.activation(cp[:], sss[half][:], AF.Copy)
                    fxp = pfx.tile([HALF, BLOC], F32, tag="fxp",
                                   name=f"fxp{phase}")
                    nc.tensor.matmul(
                        fxp[:],
                        lhsT=selcmb_sb[:],
                        rhs=cp[:],
                        start=True, stop=True,
                        skip_group_check=True,
                    )
                    recx = fxpool.tile([HALF, BLOC], F32, tag="recx",
                                       name=f"recx{phase}")
                    nc.vector.reciprocal_approx_fast(recx[:], fxp[:])
                    fx = fxpool.tile([HALF, BLOC], BF16, tag="fx",
                                     name=f"fx{phase}")
                    nc.scalar.activation(fx[:], recx[:], AF.Sqrt)
                    fx_sbs[phase] = fx

                def new_lg(k):
                    lg_tiles[k] = plog.tile([C, BLOC], F32, tag="lg",
                                            name=f"lg{k}")

                # HAM warmup: ~3.2us of tiny junk matmuls on the const
                # tiles (land ~8.5us) so the real stream runs at 2.4GHz
                junk = pfb.tile([HALF, HALF], F32, tag="junk", name="junk")
                for _ in range(36):
                    nc.tensor.matmul(
                        junk[:], lhsT=selk8_sb[:, 0, :], rhs=selcmb_sb[:],
                        start=True, stop=True, skip_group_check=True,
                    )

                # k=0: mains in DMA-piece order (earliest PE start), then ss
                emit_squares(0, 0, 1)
                new_lg(0)
                mains(0, 0, 2)
                emit_squares(0, 1, 4)
                mains(0, 2, DC)
                ssmm(0)
                for k in range(1, K - 1):
                    emit_squares(k)
                    new_lg(k)
                    mains(k, 0, DC)
                    ssmm(k)
                    emit_drain(k - 1)
                    if k == HALF - 1:
                        emit_combine(0, "a")
                    if k == K - 2:
                        emit_combine(1, "b")
                    if k >= HALF:
                        emit_output_stage(k - HALF, "a")
                    if k == HALF + 1:
                        emit_output_stage(HALF - 1, "a")

                # k=7: ss pack first; the tiny slot-3 combine slots between
                # the mains so only out7's chain trails the last matmul
                k = K - 1
                emit_drain(k - 1)
                emit_squares(k)
                new_lg(k)
                emit_output_stage(HALF, "b", nrows=3)
                emit_output_stage(HALF + 1, "b", nrows=3)
                emit_output_stage(HALF + 2, "b", nrows=3)
                ssmm(k)
                mains(k, 0, 1)
                emit_combine(1, "c")
                mains(k, 1, 6)
                # k7 output off the critical chain: fb7 early (fx_c lands
                # mid-mains), drain the SMALL fb on ACT during the last
                # mains, then mul7 reads lg7 straight from PSUM -- no lg
                # drain in the tail
                fb7 = pfb.tile([C, BLOC], F32, tag="fb", name="fb7")
                nc.tensor.matmul(
                    fb7[:], lhsT=selc4_sb[:, HALF - 1, :],
                    rhs=fx_sbs["c"][:], start=True, stop=True,
                    skip_group_check=True,
                )
                fbd = opool.tile([C, BLOC], BF16, tag="osb", name="fbd7")
                nc.scalar.activation(fbd[:], fb7[:], AF.Copy)
                mains(k, 6, DC)
                o7 = opool.tile([C, BLOC], BF16, tag="osb", name="o7")
                nc.vector.tensor_mul(o7[:], lg_tiles[k][:], fbd[:])
                nc.sync.dma_start(out[k], o7[:])

    nc.compile()
    return nc


def _host_prep(pooled, active_expert_indices, weights, log_scales):
    idx = np.asarray(active_expert_indices).astype(np.int64)
    pooled = np.asarray(pooled, dtype=np.float32)
    weights = np.asarray(weights, dtype=np.float32)
    log_scales = np.asarray(log_scales, dtype=np.float32)

    # x: [B,K,D] -> bf16 -> per-core [K, P, DC, BLOC]  (k, d, j, b)
    pb = pooled.astype(NPBF16)
    xt_all = np.ascontiguousarray(
        pb.reshape(N_CORES, BLOC, K, DC, P).transpose(0, 2, 4, 3, 1)
    )
    # w: gather -> bf16 -> fold min(exp(ls),100)/||w_bf16|| into the
    # weights themselves (re-rounded to bf16), so the drain needs no scale
    wg = weights[idx].astype(NPBF16)  # [K, C, D]
    s = np.minimum(np.exp(log_scales[idx]), 100.0).astype(np.float32)
    wn = np.sqrt((wg.astype(np.float32) ** 2).sum(-1))  # [K, C]
    rw = s[:, None] / np.maximum(wn, 1e-12)  # [K, C]
    wg = (wg.astype(np.float32) * rw[:, :, None]).astype(NPBF16)
    wt = np.ascontiguousarray(wg.reshape(K, C, DC, P).transpose(0, 3, 2, 1))

    # ss selector: one-hot k-slot column, full 128-row contraction
    selk8 = np.zeros((P, HALF, HALF), NPBF16)
    for i in range(HALF):
        selk8[:, i, i] = 1.0
    # combine: column i sums PSUM rows {32g + i}
    selcmb = np.zeros((P, HALF), NPBF16)
    for g in range(HALF):
        for i in range(HALF):
            selcmb[32 * g + i, i] = 1.0
    selc4 = np.zeros((HALF, HALF, C), NPBF16)
    for i in range(HALF):
        selc4[i, i, :] = 1.0

    shared = {"wt": wt, "selk8": selk8, "selc4": selc4, "selcmb": selcmb}
    return [dict(shared, xt=np.ascontiguousarray(xt_all[co]))
            for co in range(N_CORES)]


def kernel(pooled, active_expert_indices, weights, log_scales):
    global LAST_RESULT, LAST_WALL_NS
    if "nc" not in _CACHE:
        _CACHE["nc"] = _build()
    nc = _CACHE["nc"]

    in_maps = _host_prep(pooled, active_expert_indices, weights, log_scales)

    t0 = time.perf_counter_ns()
    res = run_bass_kernel_spmd(nc, in_maps, core_ids=list(range(N_CORES)))
    LAST_WALL_NS = time.perf_counter_ns() - t0
    LAST_RESULT = res

    full = np.stack([res.results[co]["out"] for co in range(N_CORES)])
    return np.ascontiguousarray(
        full.transpose(0, 3, 1, 2).reshape(B, K, C)
    ).astype(np.float32)
